# revision 1
# baseline (speedup 1.0000x reference)
"""DWT-based Perona-Malik diffusion block on 8 Trainium2 NeuronCores.

Math (see reference): one level of orthonormal Haar DWT, PM diffusion of the
detail subbands computed from batch 0 only and broadcast to every batch, IDWT,
then conv3x3 -> BN -> relu -> conv3x3 -> BN -> +feat.

Algebraic reductions used here:
  g   = 1 / (1 + LH^2 + HL^2)              (the sqrt cancels, K_PM = 1)
  With e = a - d and f = b - c of batch 0's 2x2 blocks (a,b,c,d = the four
  polyphase components):  LH^2 + HL^2 = (e^2 + f^2) / 2,
  u := (dLH + dHL)/2 = g*e/2,  v := (dLH - dHL)/2 = g*f/2.
  Per batch, with s1 = a + d, s2 = b + c:
    feat[2i,2j]   = s1/2 + u     feat[2i,2j+1] = s2/2 + v
    feat[2i+1,2j] = s2/2 - v     feat[2i+1,2j+1] = s1/2 - u
  BN folds into the conv weights/biases; the convs run as f32r matmuls over
  stacked (channel x row-parity) partitions, 9 taps accumulated in PSUM.

Sharding: pure data parallelism, 2 batches per core; every core redundantly
computes u,v from batch 0 (x0 is shipped to all cores).
"""

import sys

for _p in ("/opt/pypackages", "/opt/trn_rl_repo"):
    if _p not in sys.path:
        sys.path.insert(0, _p)

import numpy as np

import concourse.bass as bass
import concourse.mybir as mybir
import concourse.tile as tile_mod
from concourse.bass_utils import run_bass_kernel_spmd
from concourse.tile import TileContext
from concourse.vector_clock import ScopedClock

F32 = mybir.dt.float32
F32R = mybir.dt.float32r
BF16 = mybir.dt.bfloat16
AF = mybir.ActivationFunctionType
ALU = mybir.AluOpType

N_CORES = 8
B, C, H, W = 16, 64, 256, 256
BPC = B // N_CORES  # batches per core
NPAIR = H // 2  # 128 row pairs per image
G = 8  # row pairs per chunk
NCHUNK = NPAIR // G  # 16
EPS = 1e-5


def _patched_drain_and_barrier(self, tick_clock, wait_clock):
    # This walrus build rejects >1 sync-wait command per instruction; put the
    # tile-exit drain's waits on individual nops instead.
    nc = self.nc
    collector = nc.sync.nop(nofuse=True)
    wait_clock.add_sem_waits(
        collector.ins, ScopedClock({None: tick_clock.global_clock})
    )
    si = collector.ins.sync_info
    waits = list(si.on_wait) if si is not None else []
    if si is not None:
        si.on_wait = waits[:1]
    for w in waits[1:]:
        n = nc.sync.nop(nofuse=True)
        n.ins.sync_info = mybir.SyncInfo(on_wait=[w], on_update=[])
    nc.sync.drain()
    nc.all_engine_barrier()
    popped = nc._tile_sem_poison_stack.pop()
    assert popped is self._sem_poison
    nc.clear_and_free_semaphores(list(self.sems.allocated().values()))
    nc.all_engine_barrier()


tile_mod.TileContext._drain_and_barrier = _patched_drain_and_barrier

import concourse.bass_utils as _bu

if not getattr(_bu, "_ldw_opt_patched", False):
    _orig_run_command = _bu.run_command

    def _run_command_ldw_opt(argv, **kw):
        import os as _os

        if not _os.environ.get("NO_LDW_OPT"):
            argv = [
                "--enable-ldw-opt=true" if a == "--enable-ldw-opt=false" else a
                for a in argv
            ]
        return _orig_run_command(argv, **kw)

    _bu.run_command = _run_command_ldw_opt
    _bu._ldw_opt_patched = True


def split_multi_waits(nc):
    """Move extra sync-waits onto preceding single-wait nops (same engine)."""
    for fn in nc.m.functions:
        for blk in fn.blocks:
            new_insts = []
            for inst in blk.instructions:
                si = inst.sync_info
                waits = list(si.on_wait) if si is not None else []
                if len(waits) > 1:
                    for w in waits[:-1]:
                        n = mybir.InstNoOp(
                            name=nc.get_next_instruction_name(), ins=[], outs=[]
                        )
                        n.engine = inst.engine
                        n.bass_nofuse = True
                        n.sync_info = mybir.SyncInfo(on_wait=[w], on_update=[])
                        new_insts.append(n)
                    si.on_wait = waits[-1:]
                new_insts.append(inst)
            blk.instructions = new_insts


def _emit_conv_half(nc, psum, wts_sb, src_tile, conv, half):
    """9-tap conv for one half-chunk (2 bank-blocks): psum[128, G//2, 256].

    src_tile is [128, G+2, 258] f32r: slot t = row-pair (chunk_base - 1 + t),
    columns 1..256 are image cols 0..255, cols 0 and 257 are zero guards.
    Partition = channel + 64*(row parity); weight columns likewise for the
    output. Weight-major over the 2 blocks of this half.
    """
    base = 9 * conv
    for g, slot_off in ((0, 1), (1, 0), (2, 2)):  # A, B, C groups
        for kx in range(3):
            w_ap = wts_sb[:, base + 3 * g + kx, :]
            for b in (2 * half, 2 * half + 1):
                s0 = slot_off + 2 * b
                nc.tensor.matmul(
                    psum[:, 2 * (b - 2 * half) : 2 * (b - 2 * half) + 2, :],
                    w_ap,
                    src_tile[:, s0 : s0 + 2, kx : kx + 256],
                    start=(g == 0 and kx == 0),
                    stop=(g == 2 and kx == 2),
                )


def _build_nc():
    nc = bass.Bass("TRN2", target_bir_lowering=False, debug=False,
                   num_devices=N_CORES)

    xs_d = nc.dram_tensor("xs", [BPC, C, H, W], F32, kind="ExternalInput").ap()
    x0_d = nc.dram_tensor("x0", [C, H, W], F32, kind="ExternalInput").ap()
    wts_d = nc.dram_tensor("wts", [128, 18, 128], F32R,
                           kind="ExternalInput").ap()
    b1_d = nc.dram_tensor("bias1", [128, 1], F32, kind="ExternalInput").ap()
    b2_d = nc.dram_tensor("bias2", [128, 1], F32, kind="ExternalInput").ap()
    out_d = nc.dram_tensor("out", [BPC, C, H, W], F32,
                           kind="ExternalOutput").ap()

    GS = G + 2  # slots per chunk tile: pairs 8k-1 .. 8k+8 (self-contained)

    with TileContext(nc) as tc, nc.allow_low_precision(
        reason="PM diffusion gain g and u,v detail averages tolerate bf16"
    ):
        with (
            tc.tile_pool(name="const", bufs=1) as cpool,
            tc.tile_pool(name="uv", bufs=1) as uvpool,
            tc.tile_pool(name="xin", bufs=2) as xpool,
            tc.tile_pool(name="x0in", bufs=2) as x0pool,
            tc.tile_pool(name="dtmp", bufs=2) as dpool,
            tc.tile_pool(name="qden", bufs=1) as qpool,
            tc.tile_pool(name="fasm", bufs=2) as fpool,
            tc.tile_pool(name="featR", bufs=3) as frpool,
            tc.tile_pool(name="zbuf", bufs=2) as zpool,
            tc.tile_pool(name="outb", bufs=1) as opool,
            tc.tile_pool(name="psum1", bufs=2, space="PSUM") as p1pool,
            tc.tile_pool(name="psum2", bufs=2, space="PSUM") as p2pool,
        ):
            wts_sb = cpool.tile([128, 18, 128], F32R)
            nc.sync.dma_start(out=wts_sb[:], in_=wts_d[:])
            b1_sb = cpool.tile([128, 1], F32)
            nc.sync.dma_start(out=b1_sb[:], in_=b1_d[:])
            b2_sb = cpool.tile([128, 1], F32)
            nc.sync.dma_start(out=b2_sb[:], in_=b2_d[:])

            # u,v in "assembly" layout: partition = ch + 64*(w half),
            # free = (row-pair index, DWT col within half)
            u_t = uvpool.tile([128, NPAIR, W // 4], BF16)
            v_t = uvpool.tile([128, NPAIR, W // 4], BF16)

            featR_tiles = {}
            z_tiles = {}
            x_tiles = {}
            x0_tiles = {}

            def win(k):
                """Valid slot window [s0,s1) for chunk k (pairs 8k-1+s)."""
                s0 = 1 if k == 0 else 0
                s1 = GS - 1 if k == NCHUNK - 1 else GS
                return s0, s1

            def load_chunk(bi, k):
                s0, s1 = win(k)
                pair0 = 8 * k - 1
                rlo, rhi = 2 * (pair0 + s0), 2 * (pair0 + s1)
                xc = xpool.tile([128, GS, 2, W // 2], F32)
                x_tiles[(bi, k)] = xc
                for s in range(2):
                    nc.sync.dma_start(
                        out=xc[64 * s : 64 * (s + 1), s0:s1],
                        in_=xs_d[
                            bi, :, rlo:rhi, 128 * s : 128 * (s + 1)
                        ].rearrange("c (j r) w -> c j r w", r=2),
                    )
                if bi == 0:
                    x0c = x0pool.tile([128, GS, 2, W // 2], F32)
                    x0_tiles[k] = x0c
                    for s in range(2):
                        nc.sync.dma_start(
                            out=x0c[64 * s : 64 * (s + 1), s0:s1],
                            in_=x0_d[
                                :, rlo:rhi, 128 * s : 128 * (s + 1)
                            ].rearrange("c (j r) w -> c j r w", r=2),
                        )

            def quads(t, s0, s1):
                return (t[:, s0:s1, 0, 0::2], t[:, s0:s1, 0, 1::2],
                        t[:, s0:s1, 1, 0::2], t[:, s0:s1, 1, 1::2])

            def prep_chunk(bi, k):
                s0, s1 = win(k)
                pair0 = 8 * k - 1
                uvsl = slice(pair0 + s0, pair0 + s1)
                if bi == 0:
                    x0c = x0_tiles.pop(k)
                    a, bq, cq, d = quads(x0c, s0, s1)
                    e_t = dpool.tile([128, GS, W // 4], F32)
                    nc.vector.tensor_sub(out=e_t[:, s0:s1], in0=a, in1=d)
                    f_t = dpool.tile([128, GS, W // 4], F32)
                    nc.vector.tensor_sub(out=f_t[:, s0:s1], in0=bq, in1=cq)
                    q1_t = qpool.tile([128, GS, W // 4], F32)
                    nc.scalar.square(q1_t[:, s0:s1], e_t[:, s0:s1])
                    q2_t = qpool.tile([128, GS, W // 4], F32)
                    nc.scalar.square(q2_t[:, s0:s1], f_t[:, s0:s1])
                    den_t = qpool.tile([128, GS, W // 4], F32)
                    nc.vector.scalar_tensor_tensor(
                        out=den_t[:, s0:s1], in0=q1_t[:, s0:s1],
                        scalar=1.0, in1=q2_t[:, s0:s1],
                        op0=ALU.mult, op1=ALU.add,
                    )
                    nc.vector.tensor_scalar(
                        out=den_t[:, s0:s1], in0=den_t[:, s0:s1],
                        scalar1=0.5, scalar2=1.0,
                        op0=ALU.mult, op1=ALU.add,
                    )
                    g_t = qpool.tile([128, GS, W // 4], F32)
                    nc.vector.reciprocal(
                        out=g_t[:, s0:s1], in_=den_t[:, s0:s1]
                    )
                    nc.vector.scalar_tensor_tensor(
                        out=u_t[:, uvsl, :], in0=g_t[:, s0:s1], scalar=0.5,
                        in1=e_t[:, s0:s1], op0=ALU.mult, op1=ALU.mult,
                    )
                    nc.vector.scalar_tensor_tensor(
                        out=v_t[:, uvsl, :], in0=g_t[:, s0:s1], scalar=0.5,
                        in1=f_t[:, s0:s1], op0=ALU.mult, op1=ALU.mult,
                    )

                xc = x_tiles.pop((bi, k))
                a, bq, cq, d = quads(xc, s0, s1)
                s1_t = dpool.tile([128, GS, W // 4], F32)
                nc.vector.tensor_add(out=s1_t[:, s0:s1], in0=a, in1=d)
                s2_t = dpool.tile([128, GS, W // 4], F32)
                nc.vector.tensor_add(out=s2_t[:, s0:s1], in0=bq, in1=cq)

                fa = fpool.tile([128, GS, 2, W // 2], F32)
                for (src, uv, r, par, op1) in (
                    (s1_t, u_t, 0, 0, ALU.add),
                    (s2_t, v_t, 0, 1, ALU.add),
                    (s2_t, v_t, 1, 0, ALU.subtract),
                    (s1_t, u_t, 1, 1, ALU.subtract),
                ):
                    nc.vector.scalar_tensor_tensor(
                        out=fa[:, s0:s1, r, par::2], in0=src[:, s0:s1],
                        scalar=0.5, in1=uv[:, uvsl, :],
                        op0=ALU.mult, op1=op1,
                    )

                fr = frpool.tile([128, GS, 258], F32R)
                featR_tiles[(bi, k)] = fr
                nc.vector.memset(fr[:, s0:s1, 0:1].bitcast(F32), 0.0)
                nc.vector.memset(fr[:, s0:s1, 257:258].bitcast(F32), 0.0)
                for s in range(2):
                    for rp in range(2):
                        nc.gpsimd.dma_start(
                            out=fr[
                                64 * rp : 64 * (rp + 1), s0:s1,
                                1 + 128 * s : 1 + 128 * (s + 1),
                            ],
                            in_=fa[64 * s : 64 * (s + 1), s0:s1, rp, :],
                        )
                if k == 0:
                    nc.vector.memset(fr[:, 0:1, :].bitcast(F32), 0.0)
                if k == NCHUNK - 1:
                    nc.vector.memset(fr[:, GS - 1 : GS, :].bitcast(F32), 0.0)

            def conv1_chunk(bi, k):
                fr = featR_tiles[(bi, k)]
                zt = zpool.tile([128, GS, 258], F32R)
                z_tiles[(bi, k)] = zt
                nc.vector.memset(zt[:, 1 : G + 1, 0:1].bitcast(F32), 0.0)
                nc.vector.memset(zt[:, 1 : G + 1, 257:258].bitcast(F32), 0.0)
                for h in range(2):
                    ps1 = p1pool.tile([128, G // 2, 256], F32)
                    _emit_conv_half(nc, ps1, wts_sb, fr, 0, h)
                    nc.scalar.activation(
                        zt[:, 4 * h + 1 : 4 * h + 5, 1:257],
                        ps1[:],
                        AF.Relu, bias=b1_sb[:, 0:1], scale=1.0,
                    )
                if k == 0:
                    nc.vector.memset(zt[:, 0:1, :].bitcast(F32), 0.0)
                else:
                    zprev = z_tiles[(bi, k - 1)]
                    nc.sync.dma_start(
                        out=zt[:, 0:1, :], in_=zprev[:, G : G + 1, :]
                    )
                    nc.sync.dma_start(
                        out=zprev[:, G + 1 : G + 2, :], in_=zt[:, 1:2, :]
                    )
                if k == NCHUNK - 1:
                    nc.vector.memset(
                        zt[:, G + 1 : G + 2, :].bitcast(F32), 0.0
                    )

            def conv2_chunk(bi, n):
                ot = opool.tile([128, G, 256], F32)
                for h in range(2):
                    ps2 = p2pool.tile([128, G // 2, 256], F32)
                    _emit_conv_half(nc, ps2, wts_sb, z_tiles[(bi, n)], 1, h)
                    nc.vector.scalar_tensor_tensor(
                        out=ot[:, 4 * h : 4 * h + 4, :], in0=ps2[:],
                        scalar=b2_sb[:, 0:1],
                        in1=featR_tiles[(bi, n)][
                            :, 4 * h + 1 : 4 * h + 5, 1:257
                        ],
                        op0=ALU.add, op1=ALU.add,
                    )
                orows = slice(2 * G * n, 2 * G * (n + 1))
                for rp in range(2):
                    nc.sync.dma_start(
                        out=out_d[bi, :, orows, :].rearrange(
                            "c (j r) w -> c j r w", r=2
                        )[:, :, rp, :],
                        in_=ot[64 * rp : 64 * (rp + 1)],
                    )
                del z_tiles[(bi, n)]
                del featR_tiles[(bi, n)]

            for bi in range(BPC):
                for k in range(-1, NCHUNK + 1):
                    if 0 <= k + 1 < NCHUNK:
                        load_chunk(bi, k + 1)
                    if 0 <= k < NCHUNK:
                        prep_chunk(bi, k)
                    if 0 <= k < NCHUNK:
                        conv1_chunk(bi, k)
                    if 0 <= k - 1 < NCHUNK:
                        conv2_chunk(bi, k - 1)

    split_multi_waits(nc)
    return nc


_NC_CACHE = {}


def _get_nc():
    if "nc" not in _NC_CACHE:
        _NC_CACHE["nc"] = _build_nc()
    return _NC_CACHE["nc"]


def _host_prep(w1, b1, g1, be1, m1, v1, w2, b2, g2, be2, m2, v2):
    inv1 = (g1 / np.sqrt(v1 + EPS)).astype(np.float64)
    inv2 = (g2 / np.sqrt(v2 + EPS)).astype(np.float64)
    wc1 = w1.astype(np.float64) * inv1[:, None, None, None]
    wc2 = w2.astype(np.float64) * inv2[:, None, None, None]
    b1p = (be1.astype(np.float64) + (b1.astype(np.float64) - m1) * inv1)
    b2p = (be2.astype(np.float64) + (b2.astype(np.float64) - m2) * inv2)

    wts = np.zeros((128, 18, 128), np.float32)
    for conv, wc in ((0, wc1), (1, wc2)):
        base = 9 * conv
        for rp in range(2):
            for orp in range(2):
                ky = 1 + rp - orp
                for kx in range(3):
                    wts[
                        64 * rp : 64 * (rp + 1),
                        base + kx,
                        64 * orp : 64 * (orp + 1),
                    ] = wc[:, :, ky, kx].T.astype(np.float32)
        for kx in range(3):
            wts[64:128, base + 3 + kx, 0:64] = wc[:, :, 0, kx].T.astype(
                np.float32
            )
            wts[0:64, base + 6 + kx, 64:128] = wc[:, :, 2, kx].T.astype(
                np.float32
            )
    bias1 = np.tile(b1p.astype(np.float32), 2).reshape(128, 1)
    bias2 = np.tile(b2p.astype(np.float32), 2).reshape(128, 1)
    return wts, bias1, bias2


def kernel(x, w1, b1, g1, be1, m1, v1, w2, b2, g2, be2, m2, v2, **_kw):
    x = np.ascontiguousarray(np.asarray(x, dtype=np.float32))
    wts, bias1, bias2 = _host_prep(
        np.asarray(w1), np.asarray(b1), np.asarray(g1), np.asarray(be1),
        np.asarray(m1), np.asarray(v1), np.asarray(w2), np.asarray(b2),
        np.asarray(g2), np.asarray(be2), np.asarray(m2), np.asarray(v2),
    )
    x0 = np.ascontiguousarray(x[0])
    in_maps = []
    for c in range(N_CORES):
        in_maps.append(
            {
                "xs": np.ascontiguousarray(x[BPC * c : BPC * (c + 1)]),
                "x0": x0,
                "wts": wts,
                "bias1": bias1,
                "bias2": bias2,
            }
        )
    nc = _get_nc()
    try:
        res = run_bass_kernel_spmd(nc, in_maps, list(range(N_CORES)))
    except Exception:
        import time as _time

        _time.sleep(5)
        res = run_bass_kernel_spmd(nc, in_maps, list(range(N_CORES)))
    out = np.concatenate([r["out"] for r in res.results], axis=0)
    return out



# revision 2
# speedup vs baseline: 3.3387x; 3.3387x over previous
"""DWT-based Perona-Malik block on 8 NeuronCores — v3.

Layout (single partition scheme end-to-end): p = ch + 64*wh, wh = image
W-half. Spatial dims live in free dims with a 2-col halo per half so the
3x3 convs never cross partitions. 32-row self-contained chunks: conv1
computes z rows 32k-1..32k+32 (the 2 halo rows redundantly), so conv2 of a
chunk needs nothing from neighbor chunks except the in-tile data — no
cross-chunk z exchange, no PE-gating copy chains.

  xc [128, 18, 2, 132] x pairs 16k-1..16k+16, img cols 128wh-2..128wh+129 (f32)
  fr [128, 36, 132]    feat rows 32k-2..32k+33, same col window (bf16)
  zt [128, 34, 130]    z rows 32k-1..32k+32, img cols 128wh-1..128wh+128 (bf16)
  ot [128, 32, 128]    out rows 32k..32k+31 (f32)

All matmuls bf16 (weights + moving): same 1 cycle/row as f32r but half the
SBUF. Conv tap order for conv2 puts kx=1 (no col-halo) first so the
cross-half col copies (DVE) finish under the first taps.

Math: see reference. g = 1/(1 + (e^2+f^2)/2), e = a-d, f = b-c of batch-0
2x2 blocks; u = g*e/2, v = g*f/2; feat quad = (s1/2±u, s2/2±v); BN folded
into conv weights/biases on host. Sharding: data-parallel, 2 batches/core;
x0 shipped to all cores for the (replicated) diffusion-gain computation.
"""

import sys

for _p in ("/opt/pypackages", "/opt/trn_rl_repo"):
    if _p not in sys.path:
        sys.path.insert(0, _p)

import numpy as np

import concourse.bass as bass
import concourse.mybir as mybir
import concourse.tile as tile_mod
from concourse.bass_utils import run_bass_kernel_spmd
from concourse.tile import TileContext
from concourse.vector_clock import ScopedClock

F32 = mybir.dt.float32
F32R = mybir.dt.float32r
BF16 = mybir.dt.bfloat16
AF = mybir.ActivationFunctionType
ALU = mybir.AluOpType

N_CORES = 8
B, C, H, W = 16, 64, 256, 256
BPC = B // N_CORES
G = 16  # row pairs per chunk (32 rows)
EPS = 1e-5


def _patched_drain_and_barrier(self, tick_clock, wait_clock):
    nc = self.nc
    collector = nc.sync.nop(nofuse=True)
    wait_clock.add_sem_waits(
        collector.ins, ScopedClock({None: tick_clock.global_clock})
    )
    si = collector.ins.sync_info
    waits = list(si.on_wait) if si is not None else []
    if si is not None:
        si.on_wait = waits[:1]
    for w in waits[1:]:
        n = nc.sync.nop(nofuse=True)
        n.ins.sync_info = mybir.SyncInfo(on_wait=[w], on_update=[])
    nc.sync.drain()
    nc.all_engine_barrier()
    popped = nc._tile_sem_poison_stack.pop()
    assert popped is self._sem_poison
    nc.clear_and_free_semaphores(list(self.sems.allocated().values()))
    nc.all_engine_barrier()


tile_mod.TileContext._drain_and_barrier = _patched_drain_and_barrier

# NOTE: the baseline's forced --enable-ldw-opt=true is NOT applied here —
# walrus rejects it for bf16 LDWEIGHTS ("InstLdweights is not compatible
# with LDW optimization"); bf16 weights take the FWL fast-load path anyway.


def split_multi_waits(nc):
    for fn in nc.m.functions:
        for blk in fn.blocks:
            new_insts = []
            for inst in blk.instructions:
                si = inst.sync_info
                waits = list(si.on_wait) if si is not None else []
                if len(waits) > 1:
                    for w in waits[:-1]:
                        n = mybir.InstNoOp(
                            name=nc.get_next_instruction_name(), ins=[], outs=[]
                        )
                        n.engine = inst.engine
                        n.bass_nofuse = True
                        n.sync_info = mybir.SyncInfo(on_wait=[w], on_update=[])
                        new_insts.append(n)
                    si.on_wait = waits[-1:]
                new_insts.append(inst)
            blk.instructions = new_insts


# conv1 z-row tiles: (start_row_i, nrows) with i relative to chunk top
# (i = -1 .. 32); conv2 out-row tiles i = 0..31.
C1_TILES = [(-1, 4), (3, 4), (7, 4), (11, 4), (15, 4), (19, 4), (23, 4),
            (27, 4), (31, 2)]
C2_TILES = [(0, 4), (4, 4), (8, 4), (12, 4), (16, 4), (20, 4), (24, 4),
            (28, 4)]
# kx=1 (center col, no halo) taps first so col-halo copies don't gate PE
TAPS2 = [(ky, 1) for ky in range(3)] + [(ky, 0) for ky in range(3)] + [
    (ky, 2) for ky in range(3)]
TAPS1 = [(ky, kx) for ky in range(3) for kx in range(3)]


def _build_nc(bpc=BPC, h=H, reps=1):
    npair = h // 2
    nchunk = npair // G
    GS = G + 2

    nc = bass.Bass("TRN2", target_bir_lowering=False, debug=False,
                   num_devices=N_CORES)

    xs_d = nc.dram_tensor("xs", [bpc, C, h, W], F32, kind="ExternalInput").ap()
    x0_d = nc.dram_tensor("x0", [C, h, W], F32, kind="ExternalInput").ap()
    wts_d = nc.dram_tensor("wts", [128, 18, 128], BF16,
                           kind="ExternalInput").ap()
    b1_d = nc.dram_tensor("bias1", [128, 1], F32, kind="ExternalInput").ap()
    b2_d = nc.dram_tensor("bias2", [128, 1], F32, kind="ExternalInput").ap()
    out_d = nc.dram_tensor("out", [bpc, C, h, W], F32,
                           kind="ExternalOutput").ap()

    def half_views(t):
        v0 = t[:, :, 0:130].rearrange("c (j r) w -> c j r w", r=2)
        v1 = t[:, :, W - 130 : W].rearrange("c (j r) w -> c j r w", r=2)
        return v0, v1

    xs_v = [half_views(xs_d[bi]) for bi in range(bpc)]
    x0_v = half_views(x0_d)

    def win(k):
        s0 = 1 if k == 0 else 0
        s1 = GS - 1 if k == nchunk - 1 else GS
        return s0, s1

    with TileContext(nc) as tc, nc.allow_low_precision(
        reason="bf16 convs and PM gain: tol 2e-2, bf16 path measured 3.4e-3"
    ):
        with (
            tc.tile_pool(name="const", bufs=1) as cpool,
            tc.tile_pool(name="uv", bufs=1) as uvpool,
            tc.tile_pool(name="xin", bufs=2) as xpool,
            tc.tile_pool(name="x0in", bufs=2) as x0pool,
            tc.tile_pool(name="dtmp", bufs=2) as dpool,
            tc.tile_pool(name="qden", bufs=1) as qpool,
            tc.tile_pool(name="featR", bufs=2) as frpool,
            tc.tile_pool(name="zbuf", bufs=2) as zpool,
            tc.tile_pool(name="outb", bufs=1) as opool,
            tc.tile_pool(name="psum1", bufs=1, space="PSUM") as p1pool,
            tc.tile_pool(name="psum2", bufs=1, space="PSUM") as p2pool,
        ):
            wts_sb = cpool.tile([128, 18, 128], BF16)
            nc.sync.dma_start(out=wts_sb[:], in_=wts_d[:])
            b1_sb = cpool.tile([128, 1], F32)
            nc.sync.dma_start(out=b1_sb[:], in_=b1_d[:])
            b2_sb = cpool.tile([128, 1], F32)
            nc.sync.dma_start(out=b2_sb[:], in_=b2_d[:])

            u_t = uvpool.tile([128, npair + 2, 66], BF16)
            v_t = uvpool.tile([128, npair + 2, 66], BF16)
            nc.gpsimd.memset(u_t[:, 0:1, :], 0.0)
            nc.gpsimd.memset(v_t[:, 0:1, :], 0.0)
            nc.gpsimd.memset(u_t[:, npair + 1 :, :], 0.0)
            nc.gpsimd.memset(v_t[:, npair + 1 :, :], 0.0)

            x_tiles = {}
            x0_tiles = {}
            fr_tiles = {}
            z_tiles = {}

            def load_tile(pool, views, k):
                s0, s1 = win(k)
                p0 = G * k - 1
                sl = slice(p0 + s0, p0 + s1)
                xc = pool.tile([128, GS, 2, 132], F32)
                nc.sync.dma_start(out=xc[0:64, s0:s1, :, 2:132],
                                  in_=views[0][:, sl])
                nc.gpsimd.memset(xc[0:64, s0:s1, :, 0:2], 0.0)
                nc.sync.dma_start(out=xc[64:128, s0:s1, :, 0:130],
                                  in_=views[1][:, sl])
                nc.gpsimd.memset(xc[64:128, s0:s1, :, 130:132], 0.0)
                return xc

            def load_chunk(bi, k):
                # x0 first: the uv chain consumes it this same iteration
                if bi == 0:
                    x0_tiles[k] = load_tile(x0pool, x0_v, k)
                x_tiles[(bi, k)] = load_tile(xpool, xs_v[bi], k)

            def quads(t, s0, s1):
                return (t[:, s0:s1, 0, 0::2], t[:, s0:s1, 0, 1::2],
                        t[:, s0:s1, 1, 0::2], t[:, s0:s1, 1, 1::2])

            def uv_prep(k):
                s0, s1 = win(k)
                p0 = G * k - 1
                uvsl = slice(p0 + s0 + 1, p0 + s1 + 1)
                x0c = x0_tiles.pop(k)
                a, bq, cq, d = quads(x0c, s0, s1)
                e_t = dpool.tile([128, GS, 66], BF16)
                nc.vector.tensor_sub(out=e_t[:, s0:s1], in0=a, in1=d)
                f_t = dpool.tile([128, GS, 66], BF16)
                nc.vector.tensor_sub(out=f_t[:, s0:s1], in0=bq, in1=cq)
                q1_t = qpool.tile([128, GS, 66], BF16)
                nc.scalar.square(q1_t[:, s0:s1], e_t[:, s0:s1])
                q2_t = qpool.tile([128, GS, 66], BF16)
                nc.scalar.square(q2_t[:, s0:s1], f_t[:, s0:s1])
                den_t = qpool.tile([128, GS, 66], BF16)
                nc.vector.scalar_tensor_tensor(
                    out=den_t[:, s0:s1], in0=q1_t[:, s0:s1],
                    scalar=1.0, in1=q2_t[:, s0:s1],
                    op0=ALU.mult, op1=ALU.add,
                )
                nc.vector.tensor_scalar(
                    out=den_t[:, s0:s1], in0=den_t[:, s0:s1],
                    scalar1=0.5, scalar2=1.0,
                    op0=ALU.mult, op1=ALU.add,
                )
                g_t = qpool.tile([128, GS, 66], BF16)
                nc.vector.reciprocal(out=g_t[:, s0:s1], in_=den_t[:, s0:s1])
                nc.vector.scalar_tensor_tensor(
                    out=u_t[:, uvsl, :], in0=g_t[:, s0:s1], scalar=0.5,
                    in1=e_t[:, s0:s1], op0=ALU.mult, op1=ALU.mult,
                )
                nc.vector.scalar_tensor_tensor(
                    out=v_t[:, uvsl, :], in0=g_t[:, s0:s1], scalar=0.5,
                    in1=f_t[:, s0:s1], op0=ALU.mult, op1=ALU.mult,
                )

            def prep(bi, k):
                s0, s1 = win(k)
                p0 = G * k - 1
                uvsl = slice(p0 + s0 + 1, p0 + s1 + 1)
                xc = x_tiles.pop((bi, k))
                a, bq, cq, d = quads(xc, s0, s1)
                s1_t = dpool.tile([128, GS, 66], BF16)
                nc.vector.tensor_add(out=s1_t[:, s0:s1], in0=a, in1=d)
                s2_t = dpool.tile([128, GS, 66], BF16)
                nc.vector.tensor_add(out=s2_t[:, s0:s1], in0=bq, in1=cq)

                fr = frpool.tile([128, 2 * GS, 132], BF16)
                fr_tiles[(bi, k)] = fr
                for (src, uv, r, par, op1) in (
                    (s1_t, u_t, 0, 0, ALU.add),
                    (s2_t, v_t, 0, 1, ALU.add),
                    (s2_t, v_t, 1, 0, ALU.subtract),
                    (s1_t, u_t, 1, 1, ALU.subtract),
                ):
                    nc.vector.scalar_tensor_tensor(
                        out=fr[:, 2 * s0 + r : 2 * s1 : 2, par::2],
                        in0=src[:, s0:s1], scalar=0.5,
                        in1=uv[:, uvsl, :], op0=ALU.mult, op1=op1,
                    )
                if k == 0:
                    nc.gpsimd.memset(fr[:, 0:2, :], 0.0)
                if k == nchunk - 1:
                    nc.gpsimd.memset(fr[:, 2 * GS - 2 :, :], 0.0)

            def conv1(bi, k):
                fr = fr_tiles[(bi, k)]
                zt = zpool.tile([128, 34, 130], BF16)
                z_tiles[(bi, k)] = zt
                for gen in (C1_TILES[0:4], C1_TILES[4:8], C1_TILES[8:9]):
                    ps = [
                        p1pool.tile([128, n, 128], F32, name=f"ps1_{qi}")
                        for qi, (i0, n) in enumerate(gen)
                    ]
                    for t, (ky, kx) in enumerate(TAPS1):
                        for qi, (i0, n) in enumerate(gen):
                            nc.tensor.matmul(
                                ps[qi][:],
                                wts_sb[:, 3 * ky + kx, :],
                                fr[:, i0 + 1 + ky : i0 + 1 + ky + n,
                                   1 + kx : 129 + kx],
                                start=(t == 0),
                                stop=(t == 8),
                            )
                    for qi, (i0, n) in enumerate(gen):
                        nc.scalar.activation(
                            zt[:, i0 + 1 : i0 + 1 + n, 1:129],
                            ps[qi][:],
                            AF.Relu, bias=b1_sb[:, 0:1], scale=1.0,
                        )
                # outer-image col guards + cross-half col halo copies
                nc.gpsimd.memset(zt[0:64, :, 0:1], 0.0)
                nc.gpsimd.memset(zt[64:128, :, 129:130], 0.0)
                nc.vector.tensor_copy(
                    out=zt[0:64, :, 129:130], in_=zt[64:128, :, 1:2]
                )
                nc.vector.tensor_copy(
                    out=zt[64:128, :, 0:1], in_=zt[0:64, :, 128:129]
                )
                # image-edge z rows must be zero-pad, not conv-of-zeros+bias
                if k == 0:
                    nc.vector.memset(zt[:, 0:1, :], 0.0)
                if k == nchunk - 1:
                    nc.vector.memset(zt[:, 33:34, :], 0.0)

            def conv2_store(bi, k):
                zt = z_tiles.pop((bi, k))
                fr = fr_tiles.pop((bi, k))
                ot = opool.tile([128, 32, 128], F32)
                for gen in (C2_TILES[0:4], C2_TILES[4:8]):
                    ps = [
                        p2pool.tile([128, n, 128], F32, name=f"ps2_{qi}")
                        for qi, (i0, n) in enumerate(gen)
                    ]
                    for t, (ky, kx) in enumerate(TAPS2):
                        for qi, (i0, n) in enumerate(gen):
                            nc.tensor.matmul(
                                ps[qi][:],
                                wts_sb[:, 9 + 3 * ky + kx, :],
                                zt[:, i0 + ky : i0 + ky + n, kx : kx + 128],
                                start=(t == 0),
                                stop=(t == 8),
                            )
                    for qi, (i0, n) in enumerate(gen):
                        nc.vector.scalar_tensor_tensor(
                            out=ot[:, i0 : i0 + n, :],
                            in0=ps[qi][:], scalar=b2_sb[:, 0:1],
                            in1=fr[:, i0 + 2 : i0 + 2 + n, 2:130],
                            op0=ALU.add, op1=ALU.add,
                        )
                # stores on the Pool SWDGE queue so they never delay loads
                rows = slice(2 * G * k, 2 * G * (k + 1))
                nc.gpsimd.dma_start(out=out_d[bi, :, rows, 0:128], in_=ot[0:64])
                nc.gpsimd.dma_start(out=out_d[bi, :, rows, 128:256],
                                    in_=ot[64:128])

            # uv_prep runs 2 chunks ahead (its serial 12-op chain must not
            # gate conv1); prep runs 1 ahead so fa-STTs sit ahead of the
            # merge STTs in the DVE FIFO.
            for _rep in range(reps):
                for bi in range(bpc):
                    for k in range(-2, nchunk):
                        if 0 <= k + 1 < nchunk:
                            prep(bi, k + 1)
                        if 0 <= k + 2 < nchunk:
                            load_chunk(bi, k + 2)
                            if bi == 0:
                                uv_prep(k + 2)
                        if 0 <= k < nchunk:
                            conv1(bi, k)
                            conv2_store(bi, k)

    return nc


_NC_CACHE = {}


def _get_nc():
    if "nc" not in _NC_CACHE:
        nc = _build_nc()
        split_multi_waits(nc)
        _NC_CACHE["nc"] = nc
    return _NC_CACHE["nc"]


def _host_prep(w1, b1, g1, be1, m1, v1, w2, b2, g2, be2, m2, v2):
    import ml_dtypes

    inv1 = (g1 / np.sqrt(v1 + EPS)).astype(np.float64)
    inv2 = (g2 / np.sqrt(v2 + EPS)).astype(np.float64)
    wc1 = w1.astype(np.float64) * inv1[:, None, None, None]
    wc2 = w2.astype(np.float64) * inv2[:, None, None, None]
    b1p = (be1.astype(np.float64) + (b1.astype(np.float64) - m1) * inv1)
    b2p = (be2.astype(np.float64) + (b2.astype(np.float64) - m2) * inv2)

    wts = np.zeros((128, 18, 128), np.float32)
    for conv, wc in ((0, wc1), (1, wc2)):
        w32 = wc.astype(np.float32)
        for ky in range(3):
            for kx in range(3):
                t = 9 * conv + 3 * ky + kx
                blk = w32[:, :, ky, kx].T  # [ic, oc]
                wts[0:64, t, 0:64] = blk
                wts[64:128, t, 64:128] = blk
    wts = wts.astype(ml_dtypes.bfloat16)
    bias1 = np.tile(b1p.astype(np.float32), 2).reshape(128, 1)
    bias2 = np.tile(b2p.astype(np.float32), 2).reshape(128, 1)
    return wts, bias1, bias2


def kernel(x, w1, b1, g1, be1, m1, v1, w2, b2, g2, be2, m2, v2, **_kw):
    x = np.ascontiguousarray(np.asarray(x, dtype=np.float32))
    wts, bias1, bias2 = _host_prep(
        np.asarray(w1), np.asarray(b1), np.asarray(g1), np.asarray(be1),
        np.asarray(m1), np.asarray(v1), np.asarray(w2), np.asarray(b2),
        np.asarray(g2), np.asarray(be2), np.asarray(m2), np.asarray(v2),
    )
    x0 = np.ascontiguousarray(x[0])
    in_maps = []
    for c in range(N_CORES):
        in_maps.append(
            {
                "xs": np.ascontiguousarray(x[BPC * c : BPC * (c + 1)]),
                "x0": x0,
                "wts": wts,
                "bias1": bias1,
                "bias2": bias2,
            }
        )
    nc = _get_nc()
    try:
        res = run_bass_kernel_spmd(nc, in_maps, list(range(N_CORES)))
    except Exception:
        import time as _time

        _time.sleep(5)
        res = run_bass_kernel_spmd(nc, in_maps, list(range(N_CORES)))
    out = np.concatenate([r["out"] for r in res.results], axis=0)
    return out


# revision 3
# speedup vs baseline: 7.9756x; 2.3888x over previous
"""DWT-based Perona-Malik block on 8 NeuronCores — v3.

Layout (single partition scheme end-to-end): p = ch + 64*wh, wh = image
W-half. Spatial dims live in free dims with a 2-col halo per half so the
3x3 convs never cross partitions. 32-row self-contained chunks: conv1
computes z rows 32k-1..32k+32 (the 2 halo rows redundantly), so conv2 of a
chunk needs nothing from neighbor chunks except the in-tile data — no
cross-chunk z exchange, no PE-gating copy chains.

  xc [128, 18, 2, 132] x pairs 16k-1..16k+16, img cols 128wh-2..128wh+129 (f32)
  fr [128, 36, 132]    feat rows 32k-2..32k+33, same col window (bf16)
  zt [128, 34, 130]    z rows 32k-1..32k+32, img cols 128wh-1..128wh+128 (bf16)
  ot [128, 32, 128]    out rows 32k..32k+31 (f32)

All matmuls bf16 (weights + moving): same 1 cycle/row as f32r but half the
SBUF. Conv tap order for conv2 puts kx=1 (no col-halo) first so the
cross-half col copies (DVE) finish under the first taps.

Math: see reference. g = 1/(1 + (e^2+f^2)/2), e = a-d, f = b-c of batch-0
2x2 blocks; u = g*e/2, v = g*f/2; feat quad = (s1/2±u, s2/2±v); BN folded
into conv weights/biases on host. Sharding: data-parallel, 2 batches/core;
x0 shipped to all cores for the (replicated) diffusion-gain computation.
"""

import sys

for _p in ("/opt/pypackages", "/opt/trn_rl_repo"):
    if _p not in sys.path:
        sys.path.insert(0, _p)

import numpy as np

import concourse.bass as bass
import concourse.mybir as mybir
import concourse.tile as tile_mod
from concourse.bass_utils import run_bass_kernel_spmd
from concourse.tile import TileContext
from concourse.vector_clock import ScopedClock

F32 = mybir.dt.float32
F32R = mybir.dt.float32r
BF16 = mybir.dt.bfloat16
AF = mybir.ActivationFunctionType
ALU = mybir.AluOpType

N_CORES = 8
B, C, H, W = 16, 64, 256, 256
BPC = B // N_CORES
G = 16  # row pairs per chunk (32 rows)
EPS = 1e-5


def _patched_drain_and_barrier(self, tick_clock, wait_clock):
    nc = self.nc
    collector = nc.sync.nop(nofuse=True)
    wait_clock.add_sem_waits(
        collector.ins, ScopedClock({None: tick_clock.global_clock})
    )
    si = collector.ins.sync_info
    waits = list(si.on_wait) if si is not None else []
    if si is not None:
        si.on_wait = waits[:1]
    for w in waits[1:]:
        n = nc.sync.nop(nofuse=True)
        n.ins.sync_info = mybir.SyncInfo(on_wait=[w], on_update=[])
    nc.sync.drain()
    nc.all_engine_barrier()
    popped = nc._tile_sem_poison_stack.pop()
    assert popped is self._sem_poison
    nc.clear_and_free_semaphores(list(self.sems.allocated().values()))
    nc.all_engine_barrier()


tile_mod.TileContext._drain_and_barrier = _patched_drain_and_barrier

# NOTE: the baseline's forced --enable-ldw-opt=true is NOT applied here —
# walrus rejects it for bf16 LDWEIGHTS ("InstLdweights is not compatible
# with LDW optimization"); bf16 weights take the FWL fast-load path anyway.


def split_multi_waits(nc):
    for fn in nc.m.functions:
        for blk in fn.blocks:
            new_insts = []
            for inst in blk.instructions:
                si = inst.sync_info
                waits = list(si.on_wait) if si is not None else []
                if len(waits) > 1:
                    for w in waits[:-1]:
                        n = mybir.InstNoOp(
                            name=nc.get_next_instruction_name(), ins=[], outs=[]
                        )
                        n.engine = inst.engine
                        n.bass_nofuse = True
                        n.sync_info = mybir.SyncInfo(on_wait=[w], on_update=[])
                        new_insts.append(n)
                    si.on_wait = waits[-1:]
                new_insts.append(inst)
            blk.instructions = new_insts


# conv1 z-row tiles: (start_row_i, nrows) with i relative to chunk top
# (i = -1 .. 32); conv2 out-row tiles i = 0..31.
C1_TILES = [(-1, 4), (3, 4), (7, 4), (11, 4), (15, 4), (19, 4), (23, 4),
            (27, 4), (31, 2)]
C2_TILES = [(0, 4), (4, 4), (8, 4), (12, 4), (16, 4), (20, 4), (24, 4),
            (28, 4)]
# kx=1 (center col, no halo) taps first so col-halo copies don't gate PE
TAPS2 = [(ky, 1) for ky in range(3)] + [(ky, 0) for ky in range(3)] + [
    (ky, 2) for ky in range(3)]
TAPS1 = [(ky, kx) for ky in range(3) for kx in range(3)]


def _build_nc(bpc=BPC, h=H, reps=1):
    npair = h // 2
    nchunk = npair // G
    GS = G + 2

    nc = bass.Bass("TRN2", target_bir_lowering=False, debug=False,
                   num_devices=N_CORES)

    xs_d = nc.dram_tensor("xs", [bpc, C, h, W], F32, kind="ExternalInput").ap()
    x0_d = nc.dram_tensor("x0", [C, h, W], F32, kind="ExternalInput").ap()
    wts_d = nc.dram_tensor("wts", [128, 18, 128], BF16,
                           kind="ExternalInput").ap()
    b1_d = nc.dram_tensor("bias1", [128, 1], F32, kind="ExternalInput").ap()
    b2_d = nc.dram_tensor("bias2", [128, 1], F32, kind="ExternalInput").ap()
    out_d = nc.dram_tensor("out", [bpc, C, h, W], F32,
                           kind="ExternalOutput").ap()

    def half_views(t):
        v0 = t[:, :, 0:130].rearrange("c (j r) w -> c j r w", r=2)
        v1 = t[:, :, W - 130 : W].rearrange("c (j r) w -> c j r w", r=2)
        return v0, v1

    xs_v = [half_views(xs_d[bi]) for bi in range(bpc)]
    x0_v = half_views(x0_d)

    def win(k):
        s0 = 1 if k == 0 else 0
        s1 = GS - 1 if k == nchunk - 1 else GS
        return s0, s1

    with TileContext(nc) as tc, nc.allow_low_precision(
        reason="bf16 convs and PM gain: tol 2e-2, bf16 path measured 3.4e-3"
    ):
        with (
            tc.tile_pool(name="const", bufs=1) as cpool,
            tc.tile_pool(name="uv", bufs=1) as uvpool,
            tc.tile_pool(name="xin", bufs=2) as xpool,
            tc.tile_pool(name="x0in", bufs=2) as x0pool,
            tc.tile_pool(name="dtmp", bufs=2) as dpool,
            tc.tile_pool(name="qden", bufs=1) as qpool,
            tc.tile_pool(name="featR", bufs=2) as frpool,
            tc.tile_pool(name="zbuf", bufs=2) as zpool,
            tc.tile_pool(name="outb", bufs=1) as opool,
            tc.tile_pool(name="psum1", bufs=1, space="PSUM") as p1pool,
            tc.tile_pool(name="psum2", bufs=1, space="PSUM") as p2pool,
        ):
            wts_sb = cpool.tile([128, 18, 128], BF16)
            nc.sync.dma_start(out=wts_sb[:], in_=wts_d[:])
            b1_sb = cpool.tile([128, 1], F32)
            nc.sync.dma_start(out=b1_sb[:], in_=b1_d[:])
            b2_sb = cpool.tile([128, 1], F32)
            nc.sync.dma_start(out=b2_sb[:], in_=b2_d[:])

            u_t = uvpool.tile([128, npair + 2, 66], BF16)
            v_t = uvpool.tile([128, npair + 2, 66], BF16)
            nc.gpsimd.memset(u_t[:, 0:1, :], 0.0)
            nc.gpsimd.memset(v_t[:, 0:1, :], 0.0)
            nc.gpsimd.memset(u_t[:, npair + 1 :, :], 0.0)
            nc.gpsimd.memset(v_t[:, npair + 1 :, :], 0.0)

            x_tiles = {}
            x0_tiles = {}
            fr_tiles = {}
            z_tiles = {}

            def load_tile(pool, views, k):
                s0, s1 = win(k)
                p0 = G * k - 1
                sl = slice(p0 + s0, p0 + s1)
                xc = pool.tile([128, GS, 2, 132], F32)
                nc.sync.dma_start(out=xc[0:64, s0:s1, :, 2:132],
                                  in_=views[0][:, sl])
                nc.gpsimd.memset(xc[0:64, s0:s1, :, 0:2], 0.0)
                nc.sync.dma_start(out=xc[64:128, s0:s1, :, 0:130],
                                  in_=views[1][:, sl])
                nc.gpsimd.memset(xc[64:128, s0:s1, :, 130:132], 0.0)
                return xc

            def load_chunk(bi, k):
                # x0 first: the uv chain consumes it this same iteration
                if bi == 0:
                    x0_tiles[k] = load_tile(x0pool, x0_v, k)
                x_tiles[(bi, k)] = load_tile(xpool, xs_v[bi], k)

            def quads(t, s0, s1):
                return (t[:, s0:s1, 0, 0::2], t[:, s0:s1, 0, 1::2],
                        t[:, s0:s1, 1, 0::2], t[:, s0:s1, 1, 1::2])

            def uv_prep(k):
                s0, s1 = win(k)
                p0 = G * k - 1
                uvsl = slice(p0 + s0 + 1, p0 + s1 + 1)
                x0c = x0_tiles.pop(k)
                a, bq, cq, d = quads(x0c, s0, s1)
                e_t = dpool.tile([128, GS, 66], BF16)
                nc.gpsimd.tensor_sub(out=e_t[:, s0:s1], in0=a, in1=d)
                f_t = dpool.tile([128, GS, 66], BF16)
                nc.gpsimd.tensor_sub(out=f_t[:, s0:s1], in0=bq, in1=cq)
                q1_t = qpool.tile([128, GS, 66], BF16)
                nc.scalar.square(q1_t[:, s0:s1], e_t[:, s0:s1])
                q2_t = qpool.tile([128, GS, 66], BF16)
                nc.scalar.square(q2_t[:, s0:s1], f_t[:, s0:s1])
                den_t = qpool.tile([128, GS, 66], BF16)
                nc.vector.scalar_tensor_tensor(
                    out=den_t[:, s0:s1], in0=q1_t[:, s0:s1],
                    scalar=1.0, in1=q2_t[:, s0:s1],
                    op0=ALU.mult, op1=ALU.add,
                )
                nc.vector.tensor_scalar(
                    out=den_t[:, s0:s1], in0=den_t[:, s0:s1],
                    scalar1=0.5, scalar2=1.0,
                    op0=ALU.mult, op1=ALU.add,
                )
                g_t = qpool.tile([128, GS, 66], BF16)
                nc.vector.reciprocal(out=g_t[:, s0:s1], in_=den_t[:, s0:s1])
                nc.vector.scalar_tensor_tensor(
                    out=u_t[:, uvsl, :], in0=g_t[:, s0:s1], scalar=0.5,
                    in1=e_t[:, s0:s1], op0=ALU.mult, op1=ALU.mult,
                )
                nc.vector.scalar_tensor_tensor(
                    out=v_t[:, uvsl, :], in0=g_t[:, s0:s1], scalar=0.5,
                    in1=f_t[:, s0:s1], op0=ALU.mult, op1=ALU.mult,
                )

            def prep(bi, k):
                s0, s1 = win(k)
                p0 = G * k - 1
                uvsl = slice(p0 + s0 + 1, p0 + s1 + 1)
                xc = x_tiles.pop((bi, k))
                a, bq, cq, d = quads(xc, s0, s1)
                s1_t = dpool.tile([128, GS, 66], BF16)
                nc.gpsimd.tensor_add(out=s1_t[:, s0:s1], in0=a, in1=d)
                s2_t = dpool.tile([128, GS, 66], BF16)
                nc.gpsimd.tensor_add(out=s2_t[:, s0:s1], in0=bq, in1=cq)

                fr = frpool.tile([128, 2 * GS, 132], BF16)
                fr_tiles[(bi, k)] = fr
                for (src, uv, r, par, op1) in (
                    (s1_t, u_t, 0, 0, ALU.add),
                    (s2_t, v_t, 0, 1, ALU.add),
                    (s2_t, v_t, 1, 0, ALU.subtract),
                    (s1_t, u_t, 1, 1, ALU.subtract),
                ):
                    nc.vector.scalar_tensor_tensor(
                        out=fr[:, 2 * s0 + r : 2 * s1 : 2, par::2],
                        in0=src[:, s0:s1], scalar=0.5,
                        in1=uv[:, uvsl, :], op0=ALU.mult, op1=op1,
                    )
                if k == 0:
                    nc.gpsimd.memset(fr[:, 0:2, :], 0.0)
                if k == nchunk - 1:
                    nc.gpsimd.memset(fr[:, 2 * GS - 2 :, :], 0.0)

            def conv1(bi, k):
                fr = fr_tiles[(bi, k)]
                zt = zpool.tile([128, 34, 130], BF16)
                z_tiles[(bi, k)] = zt
                for gen in (C1_TILES[0:4], C1_TILES[4:8], C1_TILES[8:9]):
                    ps = [
                        p1pool.tile([128, n, 128], F32, name=f"ps1_{qi}")
                        for qi, (i0, n) in enumerate(gen)
                    ]
                    for t, (ky, kx) in enumerate(TAPS1):
                        for qi, (i0, n) in enumerate(gen):
                            nc.tensor.matmul(
                                ps[qi][:],
                                wts_sb[:, 3 * ky + kx, :],
                                fr[:, i0 + 1 + ky : i0 + 1 + ky + n,
                                   1 + kx : 129 + kx],
                                start=(t == 0),
                                stop=(t == 8),
                            )
                    for qi, (i0, n) in enumerate(gen):
                        nc.scalar.activation(
                            zt[:, i0 + 1 : i0 + 1 + n, 1:129],
                            ps[qi][:],
                            AF.Relu, bias=b1_sb[:, 0:1], scale=1.0,
                        )
                # outer-image col guards + cross-half col halo copies
                nc.gpsimd.memset(zt[0:64, :, 0:1], 0.0)
                nc.gpsimd.memset(zt[64:128, :, 129:130], 0.0)
                nc.vector.tensor_copy(
                    out=zt[0:64, :, 129:130], in_=zt[64:128, :, 1:2]
                )
                nc.vector.tensor_copy(
                    out=zt[64:128, :, 0:1], in_=zt[0:64, :, 128:129]
                )
                # image-edge z rows must be zero-pad, not conv-of-zeros+bias
                if k == 0:
                    nc.vector.memset(zt[:, 0:1, :], 0.0)
                if k == nchunk - 1:
                    nc.vector.memset(zt[:, 33:34, :], 0.0)

            def conv2_store(bi, k):
                zt = z_tiles.pop((bi, k))
                fr = fr_tiles.pop((bi, k))
                ot = opool.tile([128, 32, 128], F32)
                for gen in (C2_TILES[0:4], C2_TILES[4:8]):
                    ps = [
                        p2pool.tile([128, n, 128], F32, name=f"ps2_{qi}")
                        for qi, (i0, n) in enumerate(gen)
                    ]
                    for t, (ky, kx) in enumerate(TAPS2):
                        for qi, (i0, n) in enumerate(gen):
                            nc.tensor.matmul(
                                ps[qi][:],
                                wts_sb[:, 9 + 3 * ky + kx, :],
                                zt[:, i0 + ky : i0 + ky + n, kx : kx + 128],
                                start=(t == 0),
                                stop=(t == 8),
                            )
                    for qi, (i0, n) in enumerate(gen):
                        nc.vector.scalar_tensor_tensor(
                            out=ot[:, i0 : i0 + n, :],
                            in0=ps[qi][:], scalar=b2_sb[:, 0:1],
                            in1=fr[:, i0 + 2 : i0 + 2 + n, 2:130],
                            op0=ALU.add, op1=ALU.add,
                        )
                # stores on the Pool SWDGE queue so they never delay loads
                rows = slice(2 * G * k, 2 * G * (k + 1))
                nc.gpsimd.dma_start(out=out_d[bi, :, rows, 0:128], in_=ot[0:64])
                nc.gpsimd.dma_start(out=out_d[bi, :, rows, 128:256],
                                    in_=ot[64:128])

            # uv_prep runs 2 chunks ahead (its serial 12-op chain must not
            # gate conv1); prep runs 1 ahead so fa-STTs sit ahead of the
            # merge STTs in the DVE FIFO.
            for _rep in range(reps):
                for bi in range(bpc):
                    for k in range(-2, nchunk):
                        if 0 <= k + 1 < nchunk:
                            prep(bi, k + 1)
                        if 0 <= k + 2 < nchunk:
                            load_chunk(bi, k + 2)
                            if bi == 0:
                                uv_prep(k + 2)
                        if 0 <= k < nchunk:
                            conv1(bi, k)
                            conv2_store(bi, k)

    return nc


_NC_CACHE = {}


def _get_nc():
    if "nc" not in _NC_CACHE:
        nc = _build_nc()
        split_multi_waits(nc)
        _NC_CACHE["nc"] = nc
    return _NC_CACHE["nc"]


def _host_prep(w1, b1, g1, be1, m1, v1, w2, b2, g2, be2, m2, v2):
    import ml_dtypes

    inv1 = (g1 / np.sqrt(v1 + EPS)).astype(np.float64)
    inv2 = (g2 / np.sqrt(v2 + EPS)).astype(np.float64)
    wc1 = w1.astype(np.float64) * inv1[:, None, None, None]
    wc2 = w2.astype(np.float64) * inv2[:, None, None, None]
    b1p = (be1.astype(np.float64) + (b1.astype(np.float64) - m1) * inv1)
    b2p = (be2.astype(np.float64) + (b2.astype(np.float64) - m2) * inv2)

    wts = np.zeros((128, 18, 128), np.float32)
    for conv, wc in ((0, wc1), (1, wc2)):
        w32 = wc.astype(np.float32)
        for ky in range(3):
            for kx in range(3):
                t = 9 * conv + 3 * ky + kx
                blk = w32[:, :, ky, kx].T  # [ic, oc]
                wts[0:64, t, 0:64] = blk
                wts[64:128, t, 64:128] = blk
    wts = wts.astype(ml_dtypes.bfloat16)
    bias1 = np.tile(b1p.astype(np.float32), 2).reshape(128, 1)
    bias2 = np.tile(b2p.astype(np.float32), 2).reshape(128, 1)
    return wts, bias1, bias2


def kernel(x, w1, b1, g1, be1, m1, v1, w2, b2, g2, be2, m2, v2, **_kw):
    x = np.ascontiguousarray(np.asarray(x, dtype=np.float32))
    wts, bias1, bias2 = _host_prep(
        np.asarray(w1), np.asarray(b1), np.asarray(g1), np.asarray(be1),
        np.asarray(m1), np.asarray(v1), np.asarray(w2), np.asarray(b2),
        np.asarray(g2), np.asarray(be2), np.asarray(m2), np.asarray(v2),
    )
    x0 = np.ascontiguousarray(x[0])
    in_maps = []
    for c in range(N_CORES):
        in_maps.append(
            {
                "xs": np.ascontiguousarray(x[BPC * c : BPC * (c + 1)]),
                "x0": x0,
                "wts": wts,
                "bias1": bias1,
                "bias2": bias2,
            }
        )
    nc = _get_nc()
    try:
        res = run_bass_kernel_spmd(nc, in_maps, list(range(N_CORES)))
    except Exception:
        import time as _time

        _time.sleep(5)
        res = run_bass_kernel_spmd(nc, in_maps, list(range(N_CORES)))
    out = np.concatenate([r["out"] for r in res.results], axis=0)
    return out
